# revision 49
# baseline (speedup 1.0000x reference)
"""Trainium2 Bass kernel for AffinityLoss (nn_AffinityLoss_70875550318911).

Math: loss = mean over (n, a, b, l) of BCEWithLogits(aff_map, lb_map) where
aff_map[n,a,b,l] = sum_c lu[n,c,a,l]*lu[n,c,b,l] over 3x3 unfold positions a,b.

Reformulation: pairs (a,b) sharing relative offset d=(di,dj) share one
correlation map D_d[p] = sum_c logits[c,p]*logits[c,p+d]; by (a,b)<->(b,a)
symmetry only 13 offsets are needed (weight 2 except (0,0)). Border
multiplicities factorize into row weights wi(py) x col weights wj(px):

  total = sum_d sym_d * sum_p wi(py)*wj(px) * (softplus(D) - D*[labels match])
  loss  = total / (n * 81 * 382^2)

Per core: 48 image rows + 2 halo, both batches on partitions
(tile = [100p, (c,x)] bf16). SBUF access patterns must start at partition 0,
so the host pre-sends row/parity-shifted copies of the band (dy in {0,1,2} x
x-parity in {0,1}; parity copies keep bf16 operands 4B-aligned for the DVE
2x mode). The label path (rw*cw*[labels match]) is fully host-precomputed.

Engine split: DVE does products + c-sum tree + one weighting mult per
offset; ALL reductions run on the ACT engine as Copy-activations with
accum_out (free-dim sum) and per-partition row-weight scale. Column weights
are constant away from <=4 border columns, so they fold into a constant
(applied via a final per-column weight row) plus tiny strided border
corrections. softplus = relu(D) + ln(1+exp(-|D|)); Ln must come from the
"natural_log" ACT table (the merged natural_log_exp set's Ln is broken on
HW), so Ln work is batched into 3 phases costing 6 table loads total.

Each core returns an unnormalized scalar partial; host sums and scales.
"""
import os
import numpy as np
import ml_dtypes

NCORES = 8
N, C, H, W = 2, 19, 384, 384
KS = 3
BAND = H // NCORES          # 48 owned rows per core
TR = BAND + 2               # 50 tile rows per batch (owned + halo)
P = N * TR                  # 100 partitions
PD = P - 2                  # 98 partitions covered by compute ops
FD = C * W                  # 7296 free elements (c, x)
NOFF = 13
NACC = NOFF + 3 * NOFF + 3 * NOFF   # strips: dy | rl(main+2corr) | lt(...)

# (di, dj, sym): di >= 0; for di == 0 only dj >= 0. sym 2 covers (-di,-dj).
OFFSETS = [(0, 0, 1.0), (0, 1, 2.0), (0, 2, 2.0),
           (1, -2, 2.0), (1, -1, 2.0), (1, 0, 2.0), (1, 1, 2.0), (1, 2, 2.0),
           (2, -2, 2.0), (2, -1, 2.0), (2, 0, 2.0), (2, 1, 2.0), (2, 2, 2.0)]

# column-weight decomposition: cw(x) = C0 + corrections on border columns.
# groups are (offset, step, count, corr_value)
C0_BY_DJ = {-2: 1.0, -1: 2.0, 0: 3.0, 1: 2.0, 2: 1.0}
CORR_BY_DJ = {
    0: [(0, 383, 2, -2.0), (1, 381, 2, -1.0)],
    1: [(0, 382, 2, -1.0), (383, 1, 1, -2.0)],
    -1: [(1, 382, 2, -1.0), (0, 1, 1, -2.0)],
    2: [(382, 1, 2, -1.0)],
    -2: [(0, 1, 2, -1.0)],
}
# emission groups for the batched Ln phases
LN_GROUPS = [tuple(int(v) for v in g.split(':'))
             for g in os.environ.get('AFF_LN_GROUPS',
                                     '0:4,4:8,8:11,11:13').split(',')]

# offsets computed on the GPSIMD (Pool) engine instead of the DVE.
# Empty by default: under the Tile scheduler each Pool offset serializes
# against the DVE chain and is a net loss (measured +8us per offset).
POOL_OFFS = frozenset(int(x) for x in
                      os.environ.get("AFF_POOL_OFFS", "").split(",")
                      if x != "")

BF16 = ml_dtypes.bfloat16

_PROGRAM = None
LAST_RESULTS = None  # BassKernelResults of the most recent run (for profiling)


def _mult_weight(d: int, p: int, size: int = H) -> int:
    """Number of 3x3 window anchors pairing pixel p with p+d along one axis."""
    lo, hi = max(0, -d), 2 - max(d, 0)
    lo2, hi2 = max(lo, p - (size - KS)), min(hi, p)
    return max(0, hi2 - lo2 + 1)


def _strip_cols(q):
    """(dy, rl_main, rl_corr0, rl_corr1, lt_main, lt_corr0, lt_corr1)."""
    return (q,
            NOFF + 3 * q, NOFF + 3 * q + 1, NOFF + 3 * q + 2,
            4 * NOFF + 3 * q, 4 * NOFF + 3 * q + 1, 4 * NOFF + 3 * q + 2)


def _build_program():
    import concourse.tile as tile
    from concourse import bacc, mybir
    from concourse.alu_op_type import AluOpType
    from contextlib import ExitStack

    bf = mybir.dt.bfloat16
    f32 = mybir.dt.float32
    A = AluOpType
    AF = mybir.ActivationFunctionType
    Copy = AF.Copy

    nc = bacc.Bacc("TRN2", target_bir_lowering=False, debug=False,
                   num_devices=NCORES)

    lg_d = {}
    for dy in range(3):
        for par in range(2):
            lg_d[(dy, par)] = nc.dram_tensor(
                f"lg_d{dy}p{par}", [P, FD + 4], bf, kind="ExternalInput")
    # mq = rw * cw * [labels match] per offset, host-precomputed
    mq_d = [nc.dram_tensor(f"mq{q}", [P, W], bf, kind="ExternalInput")
            for q in range(NOFF)]
    rw = nc.dram_tensor("rw", [P, NOFF], f32, kind="ExternalInput")
    colw_d = nc.dram_tensor("colw", [1, NACC], f32, kind="ExternalInput")
    out = nc.dram_tensor("out", [1, 1], f32, kind="ExternalOutput")
    dbg = None
    if os.environ.get("AFF_DEBUG_STRIPS"):
        dbg = nc.dram_tensor("dbg", [P, NACC], f32, kind="ExternalOutput")

    with ExitStack() as ctx:
        tc = ctx.enter_context(tile.TileContext(nc))
        singles = ctx.enter_context(tc.tile_pool(name="singles", bufs=1))
        work = ctx.enter_context(tc.tile_pool(name="work", bufs=1))
        pipe = ctx.enter_context(tc.tile_pool(name="pipe", bufs=2))
        enp = ctx.enter_context(tc.tile_pool(name="enp", bufs=7))
        psum = ctx.enter_context(tc.tile_pool(name="psum", bufs=1, space="PSUM"))

        LG = {}
        for dy in range(3):
            for par in range(2):
                LG[(dy, par)] = singles.tile([P, FD + 4], bf, name=f"LG{dy}{par}")
        MQ = [singles.tile([P, W], bf, name=f"MQ{q}") for q in range(NOFF)]
        rwt = singles.tile([P, NOFF], f32)
        colwt = singles.tile([1, NACC], f32)
        accs = {tg: singles.tile([P, NACC], f32, name=f"acc{tg}")
                for tg in ("v", "p")}
        ones = singles.tile([P, 1], f32)

        nc.vector.memset(ones[:], 1.0)
        for tg in ("v", "p"):
            nc.vector.memset(accs[tg][:], 0.0)
        # logit tiles first (they gate compute)
        order = [(0, 0)]
        if POOL_OFFS:
            pf = OFFSETS[min(POOL_OFFS)]
            key = (pf[0], pf[1] & 1)
            if key not in order:
                order.append(key)
        for dy in range(3):
            for par in range(2):
                if (dy, par) not in order:
                    order.append((dy, par))
        # first tile in two halves so the first product can start ~3us
        # earlier (on the c 0..8 half); HSPLIT covers both operands of the
        # first half for any x-offset in {0..4}
        HSPLIT = 2 + 9 * W + 4
        first = order[0]
        nc.sync.dma_start(LG[first][:, 0:HSPLIT], lg_d[first][:, 0:HSPLIT])
        nc.sync.dma_start(LG[first][:, HSPLIT:], lg_d[first][:, HSPLIT:])
        for key in order[1:]:
            nc.sync.dma_start(LG[key][:], lg_d[key][:])
        # weights/label products on the scalar HWDGE queue (needed later)
        nc.scalar.dma_start(rwt[:], rw[:])
        nc.scalar.dma_start(colwt[:], colw_d[:])
        for q in range(NOFF):
            nc.scalar.dma_start(MQ[q][:], mq_d[q][:])

        base = LG[(0, 0)]
        dve_qs = [q for q in range(NOFF) if q not in POOL_OFFS]
        pool_qs = sorted(POOL_OFFS)
        seq = dve_qs + pool_qs
        assert sorted(seq) == list(range(NOFF)), seq

        en_tiles = {}
        act_seq = []  # ACT instrs in emission order; chained below so the
        # scheduler can't interleave Ln-table ops into exp-table phases

        def _act(*args, **kw):
            inst = nc.scalar.activation(*args, **kw)
            act_seq.append(inst)
            return inst

        def phase1(q):
            di, dj, _sym = OFFSETS[q]
            par = dj & 1
            xoff = 2 + dj - par
            on_pool = q in POOL_OFFS
            e = nc.gpsimd if on_pool else nc.vector
            tg = "p" if on_pool else "v"
            src = LG[(di, par)]
            acc = accs[tg]
            cdy, crm, crc0, crc1, _clm, _clc0, _clc1 = _strip_cols(q)

            # D_d = sum_c L * shift_d(L), bf16 tree over c (19 = 9+9+1)
            prod = work.tile([PD, FD], bf, tag="prod" + tg)
            if q == seq[0]:
                # split so the first half starts as soon as its DMA lands
                e.tensor_tensor(prod[:, 0:9 * W], base[0:PD, 2:2 + 9 * W],
                                src[0:PD, xoff:xoff + 9 * W], A.mult)
                e.tensor_tensor(prod[:, 9 * W:FD],
                                base[0:PD, 2 + 9 * W:FD + 2],
                                src[0:PD, xoff + 9 * W:xoff + FD], A.mult)
            else:
                e.tensor_tensor(prod[:], base[0:PD, 2:FD + 2],
                                src[0:PD, xoff:xoff + FD], A.mult)
            s1 = work.tile([PD, 9 * W], bf, tag="s1" + tg)
            e.tensor_tensor(s1[:], prod[:, 0:9 * W], prod[:, 9 * W:18 * W], A.add)
            s2 = work.tile([PD, 4 * W], bf, tag="s2" + tg)
            e.tensor_tensor(s2[:], s1[:, 0:4 * W], s1[:, 4 * W:8 * W], A.add)
            s3 = work.tile([PD, 2 * W], bf, tag="s3" + tg)
            e.tensor_tensor(s3[:], s2[:, 0:2 * W], s2[:, 2 * W:4 * W], A.add)
            s4 = work.tile([PD, W], bf, tag="s4" + tg)
            e.tensor_tensor(s4[:], s3[:, 0:W], s3[:, W:2 * W], A.add)
            s5 = work.tile([PD, W], bf, tag="s5" + tg)
            e.tensor_tensor(s5[:], s4[:], s1[:, 8 * W:9 * W], A.add)
            D = pipe.tile([PD, W], bf, tag="D" + tg)
            e.tensor_tensor(D[:], s5[:], prod[:, 18 * W:19 * W], A.add)

            # label term: jdy = D * mq (mq = rw*cw*[labels match]); free-dim
            # sum on ACT
            jdy = work.tile([PD, W], bf, tag="jdy" + tg)
            e.tensor_tensor(jdy[:], D[:], MQ[q][0:PD, :], A.mult)
            kdy = work.tile([PD, W], f32, tag="kdy" + tg)
            _act(kdy[:], jdy[:], Copy,
                                 accum_out=acc[0:PD, cdy:cdy + 1])

            # softplus pieces from the exp_and_others table
            ab = pipe.tile([PD, W], bf, tag="ab" + tg)
            _act(ab[:], D[:], AF.Abs)
            en = enp.tile([PD, W], f32, tag="en" + tg)
            _act(en[:], ab[:], AF.Exp, scale=-1.0)
            en_tiles[q] = en
            rl = pipe.tile([PD, W], bf, tag="rl" + tg)
            _act(rl[:], D[:], AF.Relu)
            # relu-term reductions (weights: rw scale + colw const/corr)
            krm = work.tile([PD, W], f32, tag="krm" + tg)
            _act(krm[:], rl[:], Copy, scale=rwt[0:PD, q:q + 1],
                                 accum_out=acc[0:PD, crm:crm + 1])
            for g, (off, step, cnt, _val) in enumerate(CORR_BY_DJ[dj]):
                col = crc0 if g == 0 else crc1
                kc = work.tile([PD, 2], f32, tag=f"krc{g}" + tg)
                _act(
                    kc[:, 0:cnt], rl[:, off:off + (cnt - 1) * step + 1:step],
                    Copy, scale=rwt[0:PD, q:q + 1],
                    accum_out=acc[0:PD, col:col + 1])

        def phase_ln(q):
            di, dj, _sym = OFFSETS[q]
            on_pool = q in POOL_OFFS
            tg = "p" if on_pool else "v"
            acc = accs[tg]
            _cdy, _crm, _crc0, _crc1, clm, clc0, clc1 = _strip_cols(q)
            en = en_tiles.pop(q)
            lt = pipe.tile([PD, W], bf, tag="lt" + tg)
            _act(lt[:], en[:], AF.Ln, bias=1.0)
            klm = work.tile([PD, W], f32, tag="klm" + tg)
            _act(klm[:], lt[:], Copy, scale=rwt[0:PD, q:q + 1],
                                 accum_out=acc[0:PD, clm:clm + 1])
            for g, (off, step, cnt, _val) in enumerate(CORR_BY_DJ[dj]):
                col = clc0 if g == 0 else clc1
                kc = work.tile([PD, 2], f32, tag=f"klc{g}" + tg)
                _act(
                    kc[:, 0:cnt], lt[:, off:off + (cnt - 1) * step + 1:step],
                    Copy, scale=rwt[0:PD, q:q + 1],
                    accum_out=acc[0:PD, col:col + 1])

        for lo, hi in LN_GROUPS:
            for qi in seq[lo:hi]:
                phase1(qi)
            for qi in seq[lo:hi]:
                phase_ln(qi)

        from concourse.tile import add_dep_helper
        for i in range(1, len(act_seq)):
            add_dep_helper(act_seq[i].ins, act_seq[i - 1].ins, sync=False,
                           reason="ACT emission order (table-set phases)")

        if dbg is not None:
            nc.sync.dma_start(dbg[:], accs["v"][:])
        pt = psum.tile([1, NACC], f32)
        if POOL_OFFS:
            nc.tensor.matmul(pt[:], ones[0:PD, :], accs["v"][0:PD, :],
                             start=True, stop=False)
            nc.tensor.matmul(pt[:], ones[0:PD, :], accs["p"][0:PD, :],
                             start=False, stop=True)
        else:
            nc.tensor.matmul(pt[:], ones[0:PD, :], accs["v"][0:PD, :])
        wt = singles.tile([1, NACC], f32)
        nc.vector.tensor_tensor(wt[:], pt[:], colwt[:], A.mult)
        res = singles.tile([1, 1], f32)
        nc.vector.tensor_reduce(res[:], wt[:], mybir.AxisListType.X, A.add)
        nc.sync.dma_start(out[:], res[:])
    nc.compile()
    return nc


def _host_inputs(logits: np.ndarray, labels: np.ndarray):
    logits = np.asarray(logits, dtype=np.float32)
    labels = np.asarray(labels)
    lg_bf = logits.astype(BF16).transpose(0, 2, 1, 3)   # (n, h, c, w)

    cw = np.zeros((5, W), dtype=np.float32)
    for j, dj in enumerate(range(-2, 3)):
        cw[j] = [_mult_weight(dj, px, W) for px in range(W)]
    wy_tab = np.array([[_mult_weight(d, py, H) for py in range(H)]
                       for d in range(3)], dtype=np.float32)

    # per-strip-column constants: dy -1; rl/lt main c0, corr groups their value
    colw = np.zeros((1, NACC), dtype=np.float32)
    for q, (di, dj, sym) in enumerate(OFFSETS):
        cdy, crm, crc0, crc1, clm, clc0, clc1 = _strip_cols(q)
        colw[0, cdy] = -1.0
        colw[0, crm] = colw[0, clm] = C0_BY_DJ[dj]
        for g, (_o, _s, _c, val) in enumerate(CORR_BY_DJ[dj]):
            colw[0, (crc0, crc1)[g]] = val
            colw[0, (clc0, clc1)[g]] = val

    in_maps = []
    for k in range(NCORES):
        r0 = k * BAND
        m = {}
        for dy in range(3):
            rows = max(0, min(TR, H - r0 - dy))
            band = np.zeros((N, TR, C, W), dtype=BF16)
            band[:, :rows] = lg_bf[:, r0 + dy:r0 + dy + rows, :, :]
            for par in range(2):
                if par == 0:
                    b = band
                else:
                    b = np.zeros_like(band)
                    b[..., :-1] = band[..., 1:]
                ga = np.zeros((P, FD + 4), dtype=BF16)
                ga[:, 2:FD + 2] = b.reshape(P, FD)
                m[f"lg_d{dy}p{par}"] = ga

        rwm = np.zeros((P, NOFF), dtype=np.float32)
        for q, (di, dj, sym) in enumerate(OFFSETS):
            for t in range(P):
                y = t % TR
                if y < BAND:
                    rwm[t, q] = sym * _mult_weight(di, r0 + y, H)
        m["rw"] = rwm
        m["colw"] = colw

        # mq = rw * cw * [labels match] per offset (full label path on host)
        for q, (di, dj, sym) in enumerate(OFFSETS):
            mq = np.zeros((N, TR, W), dtype=np.float32)
            rows = min(BAND, H - r0)
            py = np.arange(r0, r0 + rows)
            valid_y = py + di < H
            ys = py[valid_y]
            x0, x1 = max(0, -dj), W - max(dj, 0)
            ymask = (labels[:, ys, x0:x1] == labels[:, ys + di, x0 + dj:x1 + dj])
            wgt = (sym * wy_tab[di, ys][None, :, None]
                   * cw[dj + 2][x0:x1][None, None, :])
            mq[:, :rows][:, valid_y, x0:x1] = ymask * wgt
            m[f"mq{q}"] = mq.reshape(P, W).astype(BF16)
        in_maps.append(m)
    return in_maps


def kernel(logits: np.ndarray, labels: np.ndarray) -> np.ndarray:
    global _PROGRAM, LAST_RESULTS
    from concourse.bass_utils import run_bass_kernel_spmd

    if _PROGRAM is None:
        _PROGRAM = _build_program()

    in_maps = _host_inputs(logits, labels)
    trace = bool(int(os.environ.get("AFF_TRACE", "0")))
    results = run_bass_kernel_spmd(
        _PROGRAM, in_maps, core_ids=list(range(NCORES)), trace=trace)
    LAST_RESULTS = results

    total = 0.0
    for r in results.results:
        total += float(np.asarray(r["out"]).reshape(-1)[0])
    Lwin = (H - KS + 1) * (W - KS + 1)
    return np.float32(total / (N * KS**4 * Lwin))


# revision 50
# speedup vs baseline: 1.0040x; 1.0040x over previous
"""Trainium2 Bass kernel for AffinityLoss (nn_AffinityLoss_70875550318911).

Math: loss = mean over (n, a, b, l) of BCEWithLogits(aff_map, lb_map) where
aff_map[n,a,b,l] = sum_c lu[n,c,a,l]*lu[n,c,b,l] over 3x3 unfold positions a,b.

Reformulation: pairs (a,b) sharing relative offset d=(di,dj) share one
correlation map D_d[p] = sum_c logits[c,p]*logits[c,p+d]; by (a,b)<->(b,a)
symmetry only 13 offsets are needed (weight 2 except (0,0)). Border
multiplicities factorize into row weights wi(py) x col weights wj(px):

  total = sum_d sym_d * sum_p wi(py)*wj(px) * (softplus(D) - D*[labels match])
  loss  = total / (n * 81 * 382^2)

Per core: 48 image rows + 2 halo, both batches on partitions
(tile = [100p, (c,x)] bf16). SBUF access patterns must start at partition 0,
so the host pre-sends row/parity-shifted copies of the band (dy in {0,1,2} x
x-parity in {0,1}; parity copies keep bf16 operands 4B-aligned for the DVE
2x mode). The label path (rw*cw*[labels match]) is fully host-precomputed.

Engine split: DVE does products + c-sum tree + one weighting mult per
offset; ALL reductions run on the ACT engine as Copy-activations with
accum_out (free-dim sum) and per-partition row-weight scale. Column weights
are constant away from <=4 border columns, so they fold into a constant
(applied via a final per-column weight row) plus tiny strided border
corrections. softplus = relu(D) + ln(1+exp(-|D|)); Ln must come from the
"natural_log" ACT table (the merged natural_log_exp set's Ln is broken on
HW), so Ln work is batched into 3 phases costing 6 table loads total.

Each core returns an unnormalized scalar partial; host sums and scales.
"""
import os
import numpy as np
import ml_dtypes

NCORES = 8
N, C, H, W = 2, 19, 384, 384
KS = 3
BAND = H // NCORES          # 48 owned rows per core
TR = BAND + 2               # 50 tile rows per batch (owned + halo)
P = N * TR                  # 100 partitions
PD = P - 2                  # 98 partitions covered by compute ops
FD = C * W                  # 7296 free elements (c, x)
NOFF = 13
NACC = NOFF + 3 * NOFF + 3 * NOFF   # strips: dy | rl(main+2corr) | lt(...)

# (di, dj, sym): di >= 0; for di == 0 only dj >= 0. sym 2 covers (-di,-dj).
OFFSETS = [(0, 0, 1.0), (0, 1, 2.0), (0, 2, 2.0),
           (1, -2, 2.0), (1, -1, 2.0), (1, 0, 2.0), (1, 1, 2.0), (1, 2, 2.0),
           (2, -2, 2.0), (2, -1, 2.0), (2, 0, 2.0), (2, 1, 2.0), (2, 2, 2.0)]

# column-weight decomposition: cw(x) = C0 + corrections on border columns.
# groups are (offset, step, count, corr_value)
C0_BY_DJ = {-2: 1.0, -1: 2.0, 0: 3.0, 1: 2.0, 2: 1.0}
CORR_BY_DJ = {
    0: [(0, 383, 2, -2.0), (1, 381, 2, -1.0)],
    1: [(0, 382, 2, -1.0), (383, 1, 1, -2.0)],
    -1: [(1, 382, 2, -1.0), (0, 1, 1, -2.0)],
    2: [(382, 1, 2, -1.0)],
    -2: [(0, 1, 2, -1.0)],
}
# emission groups for the batched Ln phases
LN_GROUPS = [tuple(int(v) for v in g.split(':'))
             for g in os.environ.get('AFF_LN_GROUPS',
                                     '0:4,4:7,7:10,10:12,12:13').split(',')]

# offsets computed on the GPSIMD (Pool) engine instead of the DVE.
# Empty by default: under the Tile scheduler each Pool offset serializes
# against the DVE chain and is a net loss (measured +8us per offset).
POOL_OFFS = frozenset(int(x) for x in
                      os.environ.get("AFF_POOL_OFFS", "").split(",")
                      if x != "")

BF16 = ml_dtypes.bfloat16

_PROGRAM = None
LAST_RESULTS = None  # BassKernelResults of the most recent run (for profiling)


def _mult_weight(d: int, p: int, size: int = H) -> int:
    """Number of 3x3 window anchors pairing pixel p with p+d along one axis."""
    lo, hi = max(0, -d), 2 - max(d, 0)
    lo2, hi2 = max(lo, p - (size - KS)), min(hi, p)
    return max(0, hi2 - lo2 + 1)


def _strip_cols(q):
    """(dy, rl_main, rl_corr0, rl_corr1, lt_main, lt_corr0, lt_corr1)."""
    return (q,
            NOFF + 3 * q, NOFF + 3 * q + 1, NOFF + 3 * q + 2,
            4 * NOFF + 3 * q, 4 * NOFF + 3 * q + 1, 4 * NOFF + 3 * q + 2)


def _build_program():
    import concourse.tile as tile
    from concourse import bacc, mybir
    from concourse.alu_op_type import AluOpType
    from contextlib import ExitStack

    bf = mybir.dt.bfloat16
    f32 = mybir.dt.float32
    A = AluOpType
    AF = mybir.ActivationFunctionType
    Copy = AF.Copy

    nc = bacc.Bacc("TRN2", target_bir_lowering=False, debug=False,
                   num_devices=NCORES)

    lg_d = {}
    for dy in range(3):
        for par in range(2):
            lg_d[(dy, par)] = nc.dram_tensor(
                f"lg_d{dy}p{par}", [P, FD + 4], bf, kind="ExternalInput")
    # mq = rw * cw * [labels match] per offset, host-precomputed
    mq_d = [nc.dram_tensor(f"mq{q}", [P, W], bf, kind="ExternalInput")
            for q in range(NOFF)]
    rw = nc.dram_tensor("rw", [P, NOFF], f32, kind="ExternalInput")
    colw_d = nc.dram_tensor("colw", [1, NACC], f32, kind="ExternalInput")
    out = nc.dram_tensor("out", [1, 1], f32, kind="ExternalOutput")
    dbg = None
    if os.environ.get("AFF_DEBUG_STRIPS"):
        dbg = nc.dram_tensor("dbg", [P, NACC], f32, kind="ExternalOutput")

    with ExitStack() as ctx:
        tc = ctx.enter_context(tile.TileContext(nc))
        singles = ctx.enter_context(tc.tile_pool(name="singles", bufs=1))
        work = ctx.enter_context(tc.tile_pool(name="work", bufs=1))
        pipe = ctx.enter_context(tc.tile_pool(name="pipe", bufs=2))
        enp = ctx.enter_context(tc.tile_pool(name="enp", bufs=7))
        psum = ctx.enter_context(tc.tile_pool(name="psum", bufs=1, space="PSUM"))

        LG = {}
        for dy in range(3):
            for par in range(2):
                LG[(dy, par)] = singles.tile([P, FD + 4], bf, name=f"LG{dy}{par}")
        MQ = [singles.tile([P, W], bf, name=f"MQ{q}") for q in range(NOFF)]
        rwt = singles.tile([P, NOFF], f32)
        colwt = singles.tile([1, NACC], f32)
        accs = {tg: singles.tile([P, NACC], f32, name=f"acc{tg}")
                for tg in ("v", "p")}
        ones = singles.tile([P, 1], f32)

        nc.vector.memset(ones[:], 1.0)
        for tg in ("v", "p"):
            nc.vector.memset(accs[tg][:], 0.0)
        # logit tiles first (they gate compute)
        order = [(0, 0)]
        if POOL_OFFS:
            pf = OFFSETS[min(POOL_OFFS)]
            key = (pf[0], pf[1] & 1)
            if key not in order:
                order.append(key)
        for dy in range(3):
            for par in range(2):
                if (dy, par) not in order:
                    order.append((dy, par))
        # first tile in two halves so the first product can start ~3us
        # earlier (on the c 0..8 half); HSPLIT covers both operands of the
        # first half for any x-offset in {0..4}
        HSPLIT = 2 + 9 * W + 4
        first = order[0]
        nc.sync.dma_start(LG[first][:, 0:HSPLIT], lg_d[first][:, 0:HSPLIT])
        nc.sync.dma_start(LG[first][:, HSPLIT:], lg_d[first][:, HSPLIT:])
        for key in order[1:]:
            nc.sync.dma_start(LG[key][:], lg_d[key][:])
        # weights/label products on the scalar HWDGE queue (needed later)
        nc.scalar.dma_start(rwt[:], rw[:])
        nc.scalar.dma_start(colwt[:], colw_d[:])
        for q in range(NOFF):
            nc.scalar.dma_start(MQ[q][:], mq_d[q][:])

        base = LG[(0, 0)]
        dve_qs = [q for q in range(NOFF) if q not in POOL_OFFS]
        pool_qs = sorted(POOL_OFFS)
        seq = dve_qs + pool_qs
        assert sorted(seq) == list(range(NOFF)), seq

        en_tiles = {}
        act_seq = []  # ACT instrs in emission order; chained below so the
        # scheduler can't interleave Ln-table ops into exp-table phases

        def _act(*args, **kw):
            inst = nc.scalar.activation(*args, **kw)
            act_seq.append(inst)
            return inst

        def phase1(q):
            di, dj, _sym = OFFSETS[q]
            par = dj & 1
            xoff = 2 + dj - par
            on_pool = q in POOL_OFFS
            e = nc.gpsimd if on_pool else nc.vector
            tg = "p" if on_pool else "v"
            src = LG[(di, par)]
            acc = accs[tg]
            cdy, crm, crc0, crc1, _clm, _clc0, _clc1 = _strip_cols(q)

            # D_d = sum_c L * shift_d(L), bf16 tree over c (19 = 9+9+1)
            prod = work.tile([PD, FD], bf, tag="prod" + tg)
            if q == seq[0]:
                # split so the first half starts as soon as its DMA lands
                e.tensor_tensor(prod[:, 0:9 * W], base[0:PD, 2:2 + 9 * W],
                                src[0:PD, xoff:xoff + 9 * W], A.mult)
                e.tensor_tensor(prod[:, 9 * W:FD],
                                base[0:PD, 2 + 9 * W:FD + 2],
                                src[0:PD, xoff + 9 * W:xoff + FD], A.mult)
            else:
                e.tensor_tensor(prod[:], base[0:PD, 2:FD + 2],
                                src[0:PD, xoff:xoff + FD], A.mult)
            s1 = work.tile([PD, 9 * W], bf, tag="s1" + tg)
            e.tensor_tensor(s1[:], prod[:, 0:9 * W], prod[:, 9 * W:18 * W], A.add)
            s2 = work.tile([PD, 4 * W], bf, tag="s2" + tg)
            e.tensor_tensor(s2[:], s1[:, 0:4 * W], s1[:, 4 * W:8 * W], A.add)
            s3 = work.tile([PD, 2 * W], bf, tag="s3" + tg)
            e.tensor_tensor(s3[:], s2[:, 0:2 * W], s2[:, 2 * W:4 * W], A.add)
            s4 = work.tile([PD, W], bf, tag="s4" + tg)
            e.tensor_tensor(s4[:], s3[:, 0:W], s3[:, W:2 * W], A.add)
            s5 = work.tile([PD, W], bf, tag="s5" + tg)
            e.tensor_tensor(s5[:], s4[:], s1[:, 8 * W:9 * W], A.add)
            D = pipe.tile([PD, W], bf, tag="D" + tg)
            e.tensor_tensor(D[:], s5[:], prod[:, 18 * W:19 * W], A.add)

            # label term: jdy = D * mq (mq = rw*cw*[labels match]); free-dim
            # sum on ACT
            jdy = work.tile([PD, W], bf, tag="jdy" + tg)
            e.tensor_tensor(jdy[:], D[:], MQ[q][0:PD, :], A.mult)
            kdy = work.tile([PD, W], f32, tag="kdy" + tg)
            _act(kdy[:], jdy[:], Copy,
                                 accum_out=acc[0:PD, cdy:cdy + 1])

            # softplus pieces from the exp_and_others table
            ab = pipe.tile([PD, W], bf, tag="ab" + tg)
            _act(ab[:], D[:], AF.Abs)
            en = enp.tile([PD, W], f32, tag="en" + tg)
            _act(en[:], ab[:], AF.Exp, scale=-1.0)
            en_tiles[q] = en
            rl = pipe.tile([PD, W], bf, tag="rl" + tg)
            _act(rl[:], D[:], AF.Relu)
            # relu-term reductions (weights: rw scale + colw const/corr)
            krm = work.tile([PD, W], f32, tag="krm" + tg)
            _act(krm[:], rl[:], Copy, scale=rwt[0:PD, q:q + 1],
                                 accum_out=acc[0:PD, crm:crm + 1])
            for g, (off, step, cnt, _val) in enumerate(CORR_BY_DJ[dj]):
                col = crc0 if g == 0 else crc1
                kc = work.tile([PD, 2], f32, tag=f"krc{g}" + tg)
                _act(
                    kc[:, 0:cnt], rl[:, off:off + (cnt - 1) * step + 1:step],
                    Copy, scale=rwt[0:PD, q:q + 1],
                    accum_out=acc[0:PD, col:col + 1])

        def phase_ln(q):
            di, dj, _sym = OFFSETS[q]
            on_pool = q in POOL_OFFS
            tg = "p" if on_pool else "v"
            acc = accs[tg]
            _cdy, _crm, _crc0, _crc1, clm, clc0, clc1 = _strip_cols(q)
            en = en_tiles.pop(q)
            lt = pipe.tile([PD, W], bf, tag="lt" + tg)
            _act(lt[:], en[:], AF.Ln, bias=1.0)
            klm = work.tile([PD, W], f32, tag="klm" + tg)
            _act(klm[:], lt[:], Copy, scale=rwt[0:PD, q:q + 1],
                                 accum_out=acc[0:PD, clm:clm + 1])
            for g, (off, step, cnt, _val) in enumerate(CORR_BY_DJ[dj]):
                col = clc0 if g == 0 else clc1
                kc = work.tile([PD, 2], f32, tag=f"klc{g}" + tg)
                _act(
                    kc[:, 0:cnt], lt[:, off:off + (cnt - 1) * step + 1:step],
                    Copy, scale=rwt[0:PD, q:q + 1],
                    accum_out=acc[0:PD, col:col + 1])

        for lo, hi in LN_GROUPS:
            for qi in seq[lo:hi]:
                phase1(qi)
            for qi in seq[lo:hi]:
                phase_ln(qi)

        from concourse.tile import add_dep_helper
        for i in range(1, len(act_seq)):
            add_dep_helper(act_seq[i].ins, act_seq[i - 1].ins, sync=False,
                           reason="ACT emission order (table-set phases)")

        if dbg is not None:
            nc.sync.dma_start(dbg[:], accs["v"][:])
        pt = psum.tile([1, NACC], f32)
        if POOL_OFFS:
            nc.tensor.matmul(pt[:], ones[0:PD, :], accs["v"][0:PD, :],
                             start=True, stop=False)
            nc.tensor.matmul(pt[:], ones[0:PD, :], accs["p"][0:PD, :],
                             start=False, stop=True)
        else:
            nc.tensor.matmul(pt[:], ones[0:PD, :], accs["v"][0:PD, :])
        wt = singles.tile([1, NACC], f32)
        nc.vector.tensor_tensor(wt[:], pt[:], colwt[:], A.mult)
        res = singles.tile([1, 1], f32)
        nc.vector.tensor_reduce(res[:], wt[:], mybir.AxisListType.X, A.add)
        nc.sync.dma_start(out[:], res[:])
    nc.compile()
    return nc


def _host_inputs(logits: np.ndarray, labels: np.ndarray):
    logits = np.asarray(logits, dtype=np.float32)
    labels = np.asarray(labels)
    lg_bf = logits.astype(BF16).transpose(0, 2, 1, 3)   # (n, h, c, w)

    cw = np.zeros((5, W), dtype=np.float32)
    for j, dj in enumerate(range(-2, 3)):
        cw[j] = [_mult_weight(dj, px, W) for px in range(W)]
    wy_tab = np.array([[_mult_weight(d, py, H) for py in range(H)]
                       for d in range(3)], dtype=np.float32)

    # per-strip-column constants: dy -1; rl/lt main c0, corr groups their value
    colw = np.zeros((1, NACC), dtype=np.float32)
    for q, (di, dj, sym) in enumerate(OFFSETS):
        cdy, crm, crc0, crc1, clm, clc0, clc1 = _strip_cols(q)
        colw[0, cdy] = -1.0
        colw[0, crm] = colw[0, clm] = C0_BY_DJ[dj]
        for g, (_o, _s, _c, val) in enumerate(CORR_BY_DJ[dj]):
            colw[0, (crc0, crc1)[g]] = val
            colw[0, (clc0, clc1)[g]] = val

    in_maps = []
    for k in range(NCORES):
        r0 = k * BAND
        m = {}
        for dy in range(3):
            rows = max(0, min(TR, H - r0 - dy))
            band = np.zeros((N, TR, C, W), dtype=BF16)
            band[:, :rows] = lg_bf[:, r0 + dy:r0 + dy + rows, :, :]
            for par in range(2):
                if par == 0:
                    b = band
                else:
                    b = np.zeros_like(band)
                    b[..., :-1] = band[..., 1:]
                ga = np.zeros((P, FD + 4), dtype=BF16)
                ga[:, 2:FD + 2] = b.reshape(P, FD)
                m[f"lg_d{dy}p{par}"] = ga

        rwm = np.zeros((P, NOFF), dtype=np.float32)
        for q, (di, dj, sym) in enumerate(OFFSETS):
            for t in range(P):
                y = t % TR
                if y < BAND:
                    rwm[t, q] = sym * _mult_weight(di, r0 + y, H)
        m["rw"] = rwm
        m["colw"] = colw

        # mq = rw * cw * [labels match] per offset (full label path on host)
        for q, (di, dj, sym) in enumerate(OFFSETS):
            mq = np.zeros((N, TR, W), dtype=np.float32)
            rows = min(BAND, H - r0)
            py = np.arange(r0, r0 + rows)
            valid_y = py + di < H
            ys = py[valid_y]
            x0, x1 = max(0, -dj), W - max(dj, 0)
            ymask = (labels[:, ys, x0:x1] == labels[:, ys + di, x0 + dj:x1 + dj])
            wgt = (sym * wy_tab[di, ys][None, :, None]
                   * cw[dj + 2][x0:x1][None, None, :])
            mq[:, :rows][:, valid_y, x0:x1] = ymask * wgt
            m[f"mq{q}"] = mq.reshape(P, W).astype(BF16)
        in_maps.append(m)
    return in_maps


def kernel(logits: np.ndarray, labels: np.ndarray) -> np.ndarray:
    global _PROGRAM, LAST_RESULTS
    from concourse.bass_utils import run_bass_kernel_spmd

    if _PROGRAM is None:
        _PROGRAM = _build_program()

    in_maps = _host_inputs(logits, labels)
    trace = bool(int(os.environ.get("AFF_TRACE", "0")))
    results = run_bass_kernel_spmd(
        _PROGRAM, in_maps, core_ids=list(range(NCORES)), trace=trace)
    LAST_RESULTS = results

    total = 0.0
    for r in results.results:
        total += float(np.asarray(r["out"]).reshape(-1)[0])
    Lwin = (H - KS + 1) * (W - KS + 1)
    return np.float32(total / (N * KS**4 * Lwin))
